# revision 1
# baseline (speedup 1.0000x reference)
"""Trainium2 Bass kernel for the periodic flux-divergence stencil:

    out = sum_ax  (v - roll(v, 1, ax)),   v = 0.5*(roll(M,-1,ax)+M)*(roll(mu,-1,ax)-mu)

over axes H, W of [B=16, 1, 1024, 1024] f32 inputs, data-parallel over batch
across 8 NeuronCores (2 images per core).

Per-core plan (all-fp32, exact):
  * images tiled into 9 row-blocks of 128 rows advancing by 126 (2-row overlap
    absorbs the +-1 H-stencil taps; wrap rows come from the circular DMA load)
  * H-direction linear stencils run on TensorE as [128x128] matmuls
    (A = 0.5*(diag+superdiag), F = superdiag-diag, L = diag-subdiag)
  * W-direction shifts run on VectorE with free-dim AP offsets
  * ScalarE moves PSUM results to SBUF
"""
import sys

sys.path.insert(0, "/opt/trn_rl_repo")

import numpy as np

B, H, W = 16, 1024, 1024
N_CORES = 8
IMGS_PER_CORE = B // N_CORES          # 2
ROWS = IMGS_PER_CORE * H              # 2048
ADV = 126                             # valid rows per block
NBLK = (H + ADV - 1) // ADV           # 9
HALF = 512                            # PE moving-operand max for fp32

_CACHE = {}


def _build(reps=1, pe_final_frac=0.0, parts="wh", staggered=True):
    """pe_final_frac: fraction of half-blocks whose final t1-accumulation runs
    as an I-matmul on TensorE (plus ScalarE copy-out) instead of a VectorE add.
    parts: subset of "wh" enabling the W-direction (DVE) and H-direction (PE)
    sections — for engine-isolation benchmarking."""
    import contextlib

    import concourse.mybir as mybir
    from concourse import bacc
    from concourse.tile import TileContext

    f32 = mybir.dt.float32
    Alu = mybir.AluOpType

    nc = bacc.Bacc(trn_type="TRN2", target_bir_lowering=False)

    M_d = nc.dram_tensor("m_in", [ROWS, W], f32, kind="ExternalInput")
    MU_d = nc.dram_tensor("mu_in", [ROWS, W], f32, kind="ExternalInput")
    ST_d = nc.dram_tensor("stencils", [128, 4 * 128], f32, kind="ExternalInput")
    OUT_d = nc.dram_tensor("out", [ROWS, W], f32, kind="ExternalOutput")

    with TileContext(nc) as tc:
        with (
            tc.tile_pool(name="consts", bufs=1) as cpool,
            tc.tile_pool(name="io", bufs=3) as iopool,
            tc.tile_pool(name="work", bufs=2) as wpool,
            tc.tile_pool(name="psA", bufs=2, space="PSUM") as poolA,
            tc.tile_pool(name="psD", bufs=2, space="PSUM") as poolD,
            tc.tile_pool(name="psC", bufs=2, space="PSUM") as poolC,
        ):
            st = cpool.tile([128, 4 * 128], f32)
            nc.sync.dma_start(out=st[:], in_=ST_d[:])
            lA = st[:, 0:128]      # (0.5*(I+U)).T
            lF = st[:, 128:256]    # (U-I).T
            lL = st[:, 256:384]    # (I-D).T
            lI = st[:, 384:512]    # I

            if reps > 1 and staggered:
                loop_ctx = tc.For_i(
                    0, reps, 1,
                    staggered_reset=True,
                    hint_engines=(
                        mybir.EngineType.PE, mybir.EngineType.DVE,
                        mybir.EngineType.Activation, mybir.EngineType.SP,
                        mybir.EngineType.Pool,
                    ),
                )
            elif reps > 1:
                loop_ctx = tc.For_i(0, reps, 1)
            else:
                loop_ctx = contextlib.nullcontext()
            hb_idx = 0
            with loop_ctx:
              for img in range(IMGS_PER_CORE):
                base = img * H
                for t in range(NBLK):
                    s = (ADV * t - 1) % H
                    n1 = min(128, H - s)
                    pieces = [(0, s, n1)]
                    if n1 < 128:
                        pieces.append((n1, 0, 128 - n1))

                    mu_t = iopool.tile([128, W], f32, tag="mu")
                    m_t = iopool.tile([128, W], f32, tag="m")
                    for p0, r0, cnt in pieces:
                        nc.sync.dma_start(
                            out=mu_t[p0:p0 + cnt, :],
                            in_=MU_d[base + r0: base + r0 + cnt, :],
                        )
                        nc.sync.dma_start(
                            out=m_t[p0:p0 + cnt, :],
                            in_=M_d[base + r0: base + r0 + cnt, :],
                        )

                    out_t = wpool.tile([128, W], f32, tag="out")

                    t1 = None
                    if "w" in parts:
                        # ---- W-direction (free-dim shifts on DVE) ----
                        dw = wpool.tile([128, W], f32, tag="dw")
                        nc.vector.tensor_tensor(
                            out=dw[:, 0:W - 1], in0=mu_t[:, 1:W],
                            in1=mu_t[:, 0:W - 1], op=Alu.subtract,
                        )
                        nc.vector.tensor_tensor(
                            out=dw[:, W - 1:W], in0=mu_t[:, 0:1],
                            in1=mu_t[:, W - 1:W], op=Alu.subtract,
                        )
                        aw = wpool.tile([128, W], f32, tag="aw")
                        nc.vector.tensor_tensor(
                            out=aw[:, 0:W - 1], in0=m_t[:, 1:W],
                            in1=m_t[:, 0:W - 1], op=Alu.add,
                        )
                        nc.vector.tensor_tensor(
                            out=aw[:, W - 1:W], in0=m_t[:, 0:1],
                            in1=m_t[:, W - 1:W], op=Alu.add,
                        )
                        vw = wpool.tile([128, W], f32, tag="vw")
                        nc.vector.scalar_tensor_tensor(
                            out=vw[:], in0=aw[:], scalar=0.5, in1=dw[:],
                            op0=Alu.mult, op1=Alu.mult,
                        )
                        t1 = wpool.tile([128, W], f32, tag="t1")
                        nc.vector.tensor_tensor(
                            out=t1[:, 1:W], in0=vw[:, 1:W], in1=vw[:, 0:W - 1],
                            op=Alu.subtract,
                        )
                        nc.vector.tensor_tensor(
                            out=t1[:, 0:1], in0=vw[:, 0:1], in1=vw[:, W - 1:W],
                            op=Alu.subtract,
                        )
                        if "h" not in parts:
                            nc.vector.tensor_copy(out=out_t[:], in_=t1[:])

                    if "h" in parts:
                        # ---- H-direction (partition stencils on PE) ----
                        for h in range(2):
                            c0 = h * HALF
                            psA = poolA.tile([128, HALF], f32)
                            nc.tensor.matmul(
                                psA[:], lA, m_t[:, c0:c0 + HALF],
                                start=True, stop=True,
                            )
                            psD = poolD.tile([128, HALF], f32)
                            nc.tensor.matmul(
                                psD[:], lF, mu_t[:, c0:c0 + HALF],
                                start=True, stop=True,
                            )
                            dh_s = wpool.tile([128, HALF], f32, tag="dh")
                            nc.scalar.copy(out=dh_s[:], in_=psD[:])
                            vh = wpool.tile([128, HALF], f32, tag="vh")
                            nc.vector.tensor_tensor(
                                out=vh[:], in0=psA[:], in1=dh_s[:], op=Alu.mult
                            )
                            hb_idx += 1
                            pe_final = (
                                "w" in parts
                                and (hb_idx * pe_final_frac) % 1.0
                                >= 1.0 - pe_final_frac - 1e-9
                                and pe_final_frac > 0
                            )
                            psC = poolC.tile([128, HALF], f32)
                            if pe_final:
                                nc.tensor.matmul(
                                    psC[:], lL, vh[:], start=True, stop=False
                                )
                                nc.tensor.matmul(
                                    psC[:], lI, t1[:, c0:c0 + HALF],
                                    start=False, stop=True,
                                )
                                nc.scalar.copy(
                                    out=out_t[:, c0:c0 + HALF], in_=psC[:]
                                )
                            else:
                                nc.tensor.matmul(
                                    psC[:], lL, vh[:], start=True, stop=True
                                )
                                if "w" in parts:
                                    nc.vector.tensor_tensor(
                                        out=out_t[:, c0:c0 + HALF], in0=psC[:],
                                        in1=t1[:, c0:c0 + HALF], op=Alu.add,
                                    )
                                else:
                                    nc.scalar.copy(
                                        out=out_t[:, c0:c0 + HALF], in_=psC[:]
                                    )

                    r_out = ADV * t
                    nvalid = min(ADV, H - r_out)
                    src_t = mu_t if parts == "" else out_t
                    nc.sync.dma_start(
                        out=OUT_d[base + r_out: base + r_out + nvalid, :],
                        in_=src_t[1:1 + nvalid, :],
                    )

    nc.compile()
    return nc


def _stencils():
    st = np.zeros((128, 4 * 128), dtype=np.float32)
    A = np.zeros((128, 128), dtype=np.float32)
    F = np.zeros((128, 128), dtype=np.float32)
    L = np.zeros((128, 128), dtype=np.float32)
    for r in range(127):
        A[r, r] = 0.5
        A[r, r + 1] = 0.5
        F[r, r] = -1.0
        F[r, r + 1] = 1.0
    for r in range(1, 128):
        L[r, r] = 1.0
        L[r, r - 1] = -1.0
    st[:, 0:128] = A.T
    st[:, 128:256] = F.T
    st[:, 256:384] = L.T
    st[:, 384:512] = np.eye(128, dtype=np.float32)
    return st


def make_in_maps(inputs):
    M = np.asarray(inputs["M"], dtype=np.float32).reshape(B, H, W)
    mu = np.asarray(inputs["mu"], dtype=np.float32).reshape(B, H, W)
    st = _stencils()
    in_maps = []
    for c in range(N_CORES):
        i0 = c * IMGS_PER_CORE
        in_maps.append({
            "m_in": M[i0:i0 + IMGS_PER_CORE].reshape(ROWS, W),
            "mu_in": mu[i0:i0 + IMGS_PER_CORE].reshape(ROWS, W),
            "stencils": st,
        })
    return in_maps


def kernel(M, mu):
    from concourse.bass_utils import run_bass_kernel_spmd

    if "nc" not in _CACHE:
        _CACHE["nc"] = _build()
    nc = _CACHE["nc"]

    in_maps = make_in_maps({"M": M, "mu": mu})

    res = run_bass_kernel_spmd(nc, in_maps, core_ids=list(range(N_CORES)))
    out = np.empty((B, H, W), dtype=np.float32)
    for c in range(N_CORES):
        out[c * IMGS_PER_CORE:(c + 1) * IMGS_PER_CORE] = (
            res.results[c]["out"].reshape(IMGS_PER_CORE, H, W)
        )
    return out.reshape(B, 1, H, W)



# revision 4
# speedup vs baseline: 1.9618x; 1.9618x over previous
"""Trainium2 Bass kernel for the periodic flux-divergence stencil:

    out = sum_ax  (v - roll(v, 1, ax)),  v = 0.5*(roll(M,-1,ax)+M)*(roll(mu,-1,ax)-mu)

over axes H, W of [B=16, 1, 1024, 1024] f32 inputs, data-parallel over batch
across 8 NeuronCores (2 images per core).

v1 (fp16): host converts inputs to fp16 (halves HBM traffic, doubles DVE
throughput, quadruples PE throughput), pre-scales M by 0.5 (absorbs the
averaging factor), and pads each image row with the two circular wrap
columns (kills the [128,1] W-edge ops).

Per-core layout (host-prepared):
  m_in / mu_in: [1024, 2*1026] fp16 -- row r holds [img0 row r padded | img1
  row r padded]; padded row = [x[1023], x[0..1023], x[0]].
  out: [1024, 2*1024] fp16.

Per-iteration (9 row-blocks of 128 rows advancing by 126; the 2-row overlap
absorbs the +-1 H-stencil taps; wrap rows via the circular DMA load):
  DVE : aw = m+ + m, gw = mu+ - mu, vwu = aw*gw, t1u = Dw(vwu)  (fp16 2x)
        vhu_h = psA_h * dh_h  (PSUM 1x)
  PE  : psA = A@m (H-avg), psD = F@mu (H-diff), psC = L@vhu + 0.5I@t1u
  ACT : dh = copy(psD) fp16, out = copy(psC) fp16
  DMA : loads on SP (HWDGE), store on GPSIMD (SWDGE) for queue parallelism
"""
import sys

sys.path.insert(0, "/opt/trn_rl_repo")

import numpy as np

B, H, W = 16, 1024, 1024
N_CORES = 8
IMGS_PER_CORE = B // N_CORES          # 2
PW = W + 2                            # padded row width (wrap cols)
ADV = 126                             # valid rows per block
NBLK = (H + ADV - 1) // ADV           # 9
HALF = 512                            # PSUM bank width (f32)

_CACHE = {}


def _build():
    import contextlib

    import concourse.mybir as mybir
    from concourse import bacc
    from concourse.tile import TileContext

    f16 = mybir.dt.float16
    f32 = mybir.dt.float32
    Alu = mybir.AluOpType

    nc = bacc.Bacc(trn_type="TRN2", target_bir_lowering=False)

    M_d = nc.dram_tensor("m_in", [H, 2 * PW], f16, kind="ExternalInput")
    MU_d = nc.dram_tensor("mu_in", [H, 2 * PW], f16, kind="ExternalInput")
    ST_d = nc.dram_tensor("stencils", [128, 4 * 128], f16, kind="ExternalInput")
    OUT_d = nc.dram_tensor("out", [H, 2 * W], f16, kind="ExternalOutput")

    with TileContext(nc) as tc:
        with (
            tc.tile_pool(name="consts", bufs=1) as cpool,
            tc.tile_pool(name="io", bufs=3) as iopool,
            tc.tile_pool(name="work", bufs=2) as wpool,
            tc.tile_pool(name="psA", bufs=2, space="PSUM") as poolA,
            tc.tile_pool(name="psD", bufs=2, space="PSUM") as poolD,
            tc.tile_pool(name="psC", bufs=2, space="PSUM") as poolC,
        ):
            st = cpool.tile([128, 4 * 128], f16)
            nc.sync.dma_start(out=st[:], in_=ST_d[:])
            lA = st[:, 0:128]      # (I+U).T      H forward-average (x2)
            lF = st[:, 128:256]    # (U-I).T      H forward-diff
            lL = st[:, 256:384]    # (I-D).T      H backward-diff (divergence)
            lI = st[:, 384:512]    # I            W-part fold

            for t in range(NBLK):
                s = (ADV * t - 1) % H
                n1 = min(128, H - s)
                pieces = [(0, s, n1)]
                if n1 < 128:
                    pieces.append((n1, 0, 128 - n1))

                mu_t = iopool.tile([128, 2 * PW], f16, tag="mu")
                m_t = iopool.tile([128, 2 * PW], f16, tag="m")
                for p0, r0, cnt in pieces:
                    nc.sync.dma_start(
                        out=mu_t[p0:p0 + cnt, :],
                        in_=MU_d[r0:r0 + cnt, :],
                    )
                    nc.sync.dma_start(
                        out=m_t[p0:p0 + cnt, :],
                        in_=M_d[r0:r0 + cnt, :],
                    )

                # ---- W-direction (free-dim shifts on DVE, fp16 2x) ----
                # padded views [128, img, col]
                m3 = m_t[:].rearrange("p (j k) -> p j k", j=2)
                mu3 = mu_t[:].rearrange("p (j k) -> p j k", j=2)

                aw = wpool.tile([128, 2 * (W + 1)], f16, tag="aw")
                aw3 = aw[:].rearrange("p (j k) -> p j k", j=2)
                nc.vector.tensor_tensor(
                    out=aw3, in0=m3[:, :, 0:W + 1], in1=m3[:, :, 1:W + 2],
                    op=Alu.add,
                )
                gw = wpool.tile([128, 2 * (W + 1)], f16, tag="gw")
                gw3 = gw[:].rearrange("p (j k) -> p j k", j=2)
                nc.vector.tensor_tensor(
                    out=gw3, in0=mu3[:, :, 1:W + 2], in1=mu3[:, :, 0:W + 1],
                    op=Alu.subtract,
                )
                vwu = wpool.tile([128, 2 * (W + 1)], f16, tag="vwu")
                nc.vector.tensor_tensor(
                    out=vwu[:], in0=aw[:], in1=gw[:], op=Alu.mult,
                )
                v3 = vwu[:].rearrange("p (j k) -> p j k", j=2)
                t1u = wpool.tile([128, 2 * W], f16, tag="t1u")
                t13 = t1u[:].rearrange("p (j k) -> p j k", j=2)
                nc.vector.tensor_tensor(
                    out=t13, in0=v3[:, :, 1:W + 1], in1=v3[:, :, 0:W],
                    op=Alu.subtract,
                )

                # ---- H-direction (partition stencils on PE) ----
                dh_s = wpool.tile([128, 2 * W], f16, tag="dh")
                vhu = wpool.tile([128, 2 * W], f16, tag="vhu")
                out_t = wpool.tile([128, 2 * W], f16, tag="out")
                for pair in range(2):
                    # moving slices: central W cols of each image
                    mslc = []
                    for hh in range(2):
                        h = 2 * pair + hh
                        img, colh = divmod(h, 2)
                        c0 = img * PW + 1 + colh * HALF
                        o0 = img * W + colh * HALF
                        mslc.append((c0, o0))

                    psAs = []
                    for c0, o0 in mslc:
                        psA = poolA.tile([128, HALF], f32)
                        nc.tensor.matmul(
                            psA[:], lA, m_t[:, c0:c0 + HALF],
                            start=True, stop=True,
                        )
                        psAs.append(psA)
                    psDs = []
                    for c0, o0 in mslc:
                        psD = poolD.tile([128, HALF], f32)
                        nc.tensor.matmul(
                            psD[:], lF, mu_t[:, c0:c0 + HALF],
                            start=True, stop=True,
                        )
                        psDs.append(psD)
                    for (c0, o0), psD in zip(mslc, psDs):
                        nc.scalar.copy(
                            out=dh_s[:, o0:o0 + HALF], in_=psD[:],
                        )
                    for (c0, o0), psA in zip(mslc, psAs):
                        nc.vector.tensor_tensor(
                            out=vhu[:, o0:o0 + HALF], in0=psA[:],
                            in1=dh_s[:, o0:o0 + HALF], op=Alu.mult,
                        )
                    psCs = []
                    for c0, o0 in mslc:
                        psC = poolC.tile([128, HALF], f32)
                        nc.tensor.matmul(
                            psC[:], lL, vhu[:, o0:o0 + HALF],
                            start=True, stop=False,
                        )
                        psCs.append(psC)
                    for (c0, o0), psC in zip(mslc, psCs):
                        nc.tensor.matmul(
                            psC[:], lI, t1u[:, o0:o0 + HALF],
                            start=False, stop=True,
                        )
                    for (c0, o0), psC in zip(mslc, psCs):
                        nc.scalar.copy(
                            out=out_t[:, o0:o0 + HALF], in_=psC[:],
                        )

                r_out = ADV * t
                nvalid = min(ADV, H - r_out)
                nc.gpsimd.dma_start(
                    out=OUT_d[r_out:r_out + nvalid, :],
                    in_=out_t[1:1 + nvalid, :],
                )

    nc.compile()
    return nc


def _stencils():
    A = np.zeros((128, 128), dtype=np.float32)
    F = np.zeros((128, 128), dtype=np.float32)
    L = np.zeros((128, 128), dtype=np.float32)
    I2 = np.zeros((128, 128), dtype=np.float32)
    for r in range(127):
        A[r, r] = 1.0
        A[r, r + 1] = 1.0
        F[r, r] = -1.0
        F[r, r + 1] = 1.0
    A[127, 127] = 1.0
    F[127, 127] = -1.0
    for r in range(128):
        L[r, r] = 1.0
        I2[r, r] = 1.0
    for r in range(1, 128):
        L[r, r - 1] = -1.0
    st = np.zeros((128, 4 * 128), dtype=np.float32)
    st[:, 0:128] = A.T
    st[:, 128:256] = F.T
    st[:, 256:384] = L.T
    st[:, 384:512] = I2.T
    return st.astype(np.float16)


def _pad_rows(x):
    """[2, H, W] fp16 -> [H, 2*(W+2)] with circular wrap columns."""
    out = np.empty((H, 2, PW), dtype=np.float16)
    for j in range(2):
        out[:, j, 1:W + 1] = x[j]
        out[:, j, 0] = x[j][:, W - 1]
        out[:, j, W + 1] = x[j][:, 0]
    return out.reshape(H, 2 * PW)


def make_in_maps(inputs):
    M = np.asarray(inputs["M"], dtype=np.float32).reshape(B, H, W)
    mu = np.asarray(inputs["mu"], dtype=np.float32).reshape(B, H, W)
    st = _stencils()
    in_maps = []
    for c in range(N_CORES):
        i0 = c * IMGS_PER_CORE
        ms = (M[i0:i0 + 2] * 0.5).astype(np.float16)
        mus = mu[i0:i0 + 2].astype(np.float16)
        in_maps.append({
            "m_in": _pad_rows(ms),
            "mu_in": _pad_rows(mus),
            "stencils": st,
        })
    return in_maps


def kernel(M, mu):
    from concourse.bass_utils import run_bass_kernel_spmd

    if "nc" not in _CACHE:
        _CACHE["nc"] = _build()
    nc = _CACHE["nc"]

    in_maps = make_in_maps({"M": M, "mu": mu})

    res = run_bass_kernel_spmd(nc, in_maps, core_ids=list(range(N_CORES)))
    out = np.empty((B, H, W), dtype=np.float32)
    for c in range(N_CORES):
        o = res.results[c]["out"].reshape(H, 2, W)
        for j in range(IMGS_PER_CORE):
            out[c * IMGS_PER_CORE + j] = o[:, j, :].astype(np.float32)
    return out.reshape(B, 1, H, W)
